# revision 1
# baseline (speedup 1.0000x reference)
"""GuidedFilter (3-angle iterated boxfilter) on 8 trn2 NeuronCores.

Math: reference iterates  X <- X + (B_i(y) - B_i(X))/N_i  over 3 rotated-line
kernels B_i.  With the residual D = y - X this is  D <- D - B_i(D)/N_i,
one conv per angle, and  X_final = y - D_final.

Mapping: core (b, h) = (i//4, i%4) handles batch b, rows [512h, 512h+512).
Each core gets a 576-row slab (24-row shrink-halo per side + 8-row conv pad,
out-of-image rows zero).  Slab is processed as 5 overlapping row-chunks of
128 (stride 112).  Per angle and chunk the whole update
    Dnew = D - g_row * B'(D)        (B' = B_i / s_i, s_i = kernel sum)
is computed on the TensorEngine as 5 (or 1) banded [128,112] matmuls
(identity delta folded into the dx=2 band; row-boundary N scaling and
out-of-image masking folded into per-chunk weight variants), PSUM holds
Dnew directly, ScalarE copies PSUM->SBUF, DVE fixes the 2 leftmost /
rightmost columns (where N varies per column), DMA syncs the 8-row chunk
overlaps.  Final X = y - D on DVE, DMA out.
"""

import numpy as np

M_IMG = 2048
N_IMG = 2048
BATCH = 2
H_SHARDS = 4
SH = 512            # rows per shard
SLAB = 576          # shard + 2*32
CW = 2052           # chunk width with 2 zero-pad cols each side
NCHUNK = 5
CH_STEP = 112
KH = 17
PC = 8
PR = 2


def _host_prep(X, y, kern, N_norm):
    """Build per-core input arrays. All float32."""
    kern = np.asarray(kern, np.float64)[:, 0]        # (3,17,5)
    N = np.asarray(N_norm, np.float64)[:, 0]         # (3,2048,2048)
    D0 = (np.asarray(y) - np.asarray(X))[:, 0]       # (2,2048,2048) f32
    yf = np.asarray(y)[:, 0]

    n_ang = kern.shape[0]
    s = kern.sum(axis=(1, 2))                        # (3,)
    cols = [[dx for dx in range(kern.shape[2]) if np.abs(kern[a, :, dx]).sum() > 0]
            for a in range(n_ang)]

    # g_row(global row) = s / N(row, center col); 1.0 off-image
    grow_full = np.ones((n_ang, M_IMG), np.float64)
    for a in range(n_ang):
        grow_full[a] = s[a] / N[a, :, N_IMG // 2]

    in_maps = []
    for core in range(BATCH * H_SHARDS):
        b, h = core // H_SHARDS, core % H_SHARDS
        gs = SH * h - 32                             # global row of slab row 0

        d0s = np.zeros((SLAB, CW), np.float32)
        yss = np.zeros((SLAB, N_IMG), np.float32)
        r0, r1 = max(0, gs), min(M_IMG, gs + SLAB)
        d0s[r0 - gs:r1 - gs, 2:2 + N_IMG] = D0[b, r0:r1]
        yss[r0 - gs:r1 - gs, :] = yf[b, r0:r1]

        # --- banded weight matrices -------------------------------------
        # variant v: 0 applies to chunk 0, 1 to chunks 1..3, 2 to chunk 4.
        # PSUM chunk c partition m <-> slab row 112c+m <-> global gs+112c+m.
        # Only m in [8,120) is computable from the 128-row window; the other
        # columns stay zero and those halo partitions are refilled by the
        # chunk-overlap DMA sync.
        wts = []
        for a in range(n_ang):
            for v in range(3):
                c_of_v = {0: 0, 1: 1, 2: 4}[v]
                g_glob = gs + CH_STEP * c_of_v + np.arange(128)
                mask = (g_glob >= 0) & (g_glob < M_IMG)
                growv = np.where(mask, grow_full[a][np.clip(g_glob, 0, M_IMG - 1)], 0.0)
                for dx in cols[a]:
                    W = np.zeros((128, 128), np.float64)
                    for m in range(8, 120):
                        if mask[m]:
                            W[m - PC:m - PC + KH, m] -= growv[m] * kern[a, :, dx] / s[a]
                            if dx == 2:
                                W[m, m] += 1.0
                    wts.append(W)
        wts = np.stack(wts).astype(np.float32)       # (33,128,128)

        # --- column-strip g factors -------------------------------------
        # gc(r,c) = N(r,center)/N(r,c) for c in {0,1,2046,2047}; fix is
        # Dnew = Dold - gc*B'seen with B'seen = Dold - Dwrong.
        gcs = np.ones((n_ang, NCHUNK, 128, 4), np.float64)
        scol = [0, 1, N_IMG - 2, N_IMG - 1]
        for a in range(n_ang):
            for c in range(NCHUNK):
                g_glob = gs + CH_STEP * c + np.arange(128)   # slab row 112c+p
                ok = (g_glob >= 0) & (g_glob < M_IMG)
                gg = np.clip(g_glob, 0, M_IMG - 1)
                for j, cc in enumerate(scol):
                    v = N[a, gg, N_IMG // 2] / N[a, gg, cc]
                    gcs[a, c, :, j] = np.where(ok, v, 1.0)
        gcs = gcs.astype(np.float32)

        import ml_dtypes
        in_maps.append({"d0": d0s.astype(ml_dtypes.bfloat16), "ys": yss,
                        "wts": wts.astype(ml_dtypes.bfloat16), "gcs": gcs})

    # weight-index lookup shared by program builder
    widx = {}
    i = 0
    for a in range(n_ang):
        for v in range(3):
            for dx in cols[a]:
                widx[(a, v, dx)] = i
                i += 1
    return in_maps, cols, widx


def _build_program(cols, widx, n_w):
    import concourse.bass as bass
    from concourse import mybir

    f32 = mybir.dt.float32
    bf16 = mybir.dt.bfloat16
    nc = bass.Bass("TRN2", target_bir_lowering=False)

    d0 = nc.dram_tensor("d0", [SLAB, CW], bf16, kind="ExternalInput")
    ys = nc.dram_tensor("ys", [SLAB, N_IMG], f32, kind="ExternalInput")
    wts = nc.dram_tensor("wts", [n_w, 128, 128], bf16, kind="ExternalInput")
    gcs = nc.dram_tensor("gcs", [3, NCHUNK, 128, 4], f32, kind="ExternalInput")
    xo = nc.dram_tensor("xo", [SH, N_IMG], f32, kind="ExternalOutput")

    n_ang = len(cols)
    ping = [nc.alloc_sbuf_tensor(f"ping{c}", [128, CW], bf16) for c in range(NCHUNK)]
    pong = [nc.alloc_sbuf_tensor(f"pong{c}", [128, CW], bf16) for c in range(NCHUNK)]
    ytile = [nc.alloc_sbuf_tensor(f"yt{c}", [128, N_IMG], f32) for c in range(NCHUNK)]
    wsb = nc.alloc_sbuf_tensor("wsb", [128, n_w * 128], bf16)
    gcsb = nc.alloc_sbuf_tensor("gcsb", [128, 3 * NCHUNK * 4], f32)
    t1 = [nc.alloc_sbuf_tensor(f"t1_{c}", [128, 4], f32) for c in range(NCHUNK)]
    t2 = [nc.alloc_sbuf_tensor(f"t2_{c}", [128, 4], f32) for c in range(NCHUNK)]
    xt = [nc.alloc_sbuf_tensor(f"xt{i}", [128, N_IMG], f32) for i in range(NCHUNK)]
    ps = [nc.alloc_psum_tensor(f"ps{i}", [128, N_IMG], f32) for i in range(2)]

    def strip_ap(t):
        return bass.AP(t, 2, [[CW, 128], [N_IMG - 2, 2], [1, 2]])

    def pad_ap(t):
        return bass.AP(t, 0, [[CW, 128], [CW - 2, 2], [1, 2]])

    out_rows = [(0, 32, 120), (88, 8, 120), (200, 8, 120), (312, 8, 120), (424, 8, 96)]

    with nc.Block() as block, \
         nc.semaphore("sldw") as sldw, nc.semaphore("sldy") as sldy, nc.semaphore("spe") as spe, \
         nc.semaphore("sact") as sact, nc.semaphore("sdve") as sdve, \
         nc.semaphore("shalo") as shalo, nc.semaphore("sout") as sout, \
         nc.semaphore("sint") as sint:

        @block.sync
        def _(sp):
            sp.dma_start(out=wsb[:, :].rearrange("k (w m) -> k w m", w=n_w),
                         in_=wts[:, :, :].rearrange("w k m -> k w m")).then_inc(sldw, 16)
            sp.dma_start(out=gcsb[:, :].rearrange("k (a c j) -> k a c j", a=3, c=NCHUNK),
                         in_=gcs[:, :, :, :].rearrange("a c k j -> k a c j")).then_inc(sldw, 16)
            for c in range(NCHUNK):
                sp.dma_start(out=ping[c][:, :],
                             in_=d0[c * CH_STEP:c * CH_STEP + 128, :]).then_inc(sldw, 16)
                sp.dma_start(out=ytile[c][:, :],
                             in_=ys[c * CH_STEP:c * CH_STEP + 128, :]).then_inc(sldy, 16)
            # halo syncs for angles 0,1
            for a in range(n_ang - 1):
                dst = pong if a % 2 == 0 else ping
                for c in range(NCHUNK - 1):
                    sp.wait_ge(sdve, NCHUNK + 5 * a + c + 2)
                    sp.dma_start(out=dst[c + 1][0:8, :],
                                 in_=dst[c][112:120, :]).then_inc(shalo, 16)
                    sp.dma_start(out=dst[c][120:128, :],
                                 in_=dst[c + 1][8:16, :]).then_inc(shalo, 16)
            # output DMAs
            for c in range(NCHUNK):
                o, p0, p1 = out_rows[c]
                sp.wait_ge(sdve, NCHUNK + 3 * NCHUNK + c + 1)
                sp.dma_start(out=xo[o:o + (p1 - p0), :],
                             in_=xt[c][p0:p1, :]).then_inc(sout, 16)
            sp.wait_ge(sout, 16 * NCHUNK)

        @block.tensor
        def _(pe):
            for a in range(n_ang):
                src = ping if a % 2 == 0 else pong
                for c in range(NCHUNK):
                    g = NCHUNK * a + c
                    if a == 0:
                        if c == 0:
                            pe.wait_ge(sldw, 16 * 7)
                    else:
                        pe.wait_ge(shalo, 16 * 8 * a)
                        pe.wait_ge(sdve, g + 1)
                    if g >= 2:
                        pe.wait_ge(sact, g - 1)
                    v = {0: 0, 4: 2}.get(c, 1)
                    for nt in range(4):
                        dxs = cols[a]
                        for i, dx in enumerate(dxs):
                            wi = widx[(a, v, dx)]
                            mm = pe.matmul(ps[g % 2][:, nt * 512:(nt + 1) * 512],
                                           lhsT=wsb[:, wi * 128:(wi + 1) * 128],
                                           rhs=src[c][:, nt * 512 + dx: nt * 512 + dx + 512],
                                           start=(i == 0), stop=(i == len(dxs) - 1))
                            if nt == 3 and i == len(dxs) - 1:
                                mm.then_inc(spe, 1)

        @block.scalar
        def _(act):
            for a in range(n_ang):
                dst = pong if a % 2 == 0 else ping
                for c in range(NCHUNK):
                    g = NCHUNK * a + c
                    act.wait_ge(spe, g + 1)
                    act.copy(out=dst[c][:, 2:2 + N_IMG],
                             in_=ps[g % 2][:, :]).then_inc(sact, 1)

        @block.vector
        def _(dve):
            kint = 0
            for c in range(NCHUNK):
                dve.memset(pad_ap(pong[c]), 0.0).then_inc(sdve, 1)
            dve.wait_ge(sldw, 16 * 7)
            for a in range(n_ang):
                src = ping if a % 2 == 0 else pong
                dst = pong if a % 2 == 0 else ping
                for c in range(NCHUNK):
                    g = NCHUNK * a + c
                    dve.wait_ge(sact, g + 1)
                    gc_ap = bass.AP(gcsb, a * NCHUNK * 4 + c * 4,
                                    [[3 * NCHUNK * 4, 128], [2, 2], [1, 2]])
                    t1v = t1[c][:, :].rearrange("p (s w) -> p s w", s=2)
                    t2v = t2[c][:, :].rearrange("p (s w) -> p s w", s=2)
                    dve.tensor_sub(t1v, strip_ap(src[c]),
                                   strip_ap(dst[c])).then_inc(sint, 1)
                    kint += 1
                    dve.wait_ge(sint, kint)
                    dve.tensor_mul(t2v, t1v, gc_ap).then_inc(sint, 1)
                    kint += 1
                    dve.wait_ge(sint, kint)
                    dve.tensor_sub(strip_ap(dst[c]), strip_ap(src[c]),
                                   t2v).then_inc(sdve, 1)
            d3 = pong if (n_ang - 1) % 2 == 0 else ping
            for c in range(NCHUNK):
                if c == 0:
                    dve.wait_ge(sldy, 16 * NCHUNK)
                dve.wait_ge(sact, 2 * NCHUNK + c + 1)
                dve.tensor_sub(xt[c][:, :], ytile[c][:, :],
                               d3[c][:, 2:2 + N_IMG]).then_inc(sdve, 1)
    return nc


_LAST = None  # BassKernelResults of the most recent run (for test harness)


def kernel(X, y, kernel, N_norm):
    global _LAST
    from concourse.bass_utils import run_bass_kernel_spmd

    in_maps, cols, widx = _host_prep(X, y, kernel, N_norm)
    nc = _build_program(cols, widx, len(widx))
    res = run_bass_kernel_spmd(nc, in_maps, list(range(BATCH * H_SHARDS)))
    _LAST = res

    out = np.empty((BATCH, 1, M_IMG, N_IMG), np.float32)
    for core in range(BATCH * H_SHARDS):
        b, h = core // H_SHARDS, core % H_SHARDS
        out[b, 0, SH * h:SH * h + SH, :] = res.results[core]["xo"]
    return out



# revision 5
# speedup vs baseline: 1.6453x; 1.6453x over previous
"""GuidedFilter (3-angle iterated boxfilter) on 8 trn2 NeuronCores.

Math: reference iterates  X <- X + (B_i(y) - B_i(X))/N_i  over 3 rotated-line
kernels B_i.  With the residual D = y - X this is  D <- D - B_i(D)/N_i,
one conv per angle, and  X_final = y - D_final.

Mapping: core (b, h) = (i//4, i%4) handles batch b, rows [512h, 512h+512).
Each core gets a 576-row slab (24-row shrink-halo per side + 8-row conv pad,
out-of-image rows zero).  Slab is processed as 5 overlapping row-chunks of
128 (stride 112).  Per angle and chunk the whole update
    Dnew = D - g_row * B'(D)        (B' = B_i / s_i, s_i = kernel sum)
is computed on the TensorEngine as 5 (or 1) banded [128,112] matmuls
(identity delta folded into the dx=2 band; row-boundary N scaling and
out-of-image masking folded into per-chunk weight variants).

Pipeline (vs naive): weights/gcs are pre-transposed on host so input DMAs
are contiguous; DMA issue is split across the sync + pool queues with
angle-0 weights and the first row-chunk loaded first so the PE starts within
a few us; matmuls run dx-outer/nt-inner so each weight load feeds 4
back-to-back matmuls; PSUM->SBUF copies are split across the Scalar and
Pool engines while the DVE fixes the 4 edge columns straight out of PSUM
(no serial copy->fix chain); halo DMAs between row-chunks are gated
per-chunk so angle transitions don't barrier; the final angle skips the
SBUF copy entirely (X = y - D read straight from PSUM) and each chunk's
output DMA fires as soon as it is ready.
"""

import numpy as np

M_IMG = 2048
N_IMG = 2048
BATCH = 2
H_SHARDS = 4
SH = 512            # rows per shard
SLAB = 576          # shard + 2*32
CW = 2052           # chunk width with 2 zero-pad cols each side
NCHUNK = 5
CH_STEP = 112
KH = 17
PC = 8
PR = 2

# psum column where the scalar-engine copy ends / vector-engine copy begins
CSPLIT = 1300

# (p0, p1) valid partition range per chunk and xo row offset
OUT_ROWS = [(0, 32, 120), (88, 8, 120), (200, 8, 120), (312, 8, 120), (424, 8, 96)]


def _host_prep(X, y, kern, N_norm):
    """Build per-core input arrays. All device-ready layouts."""
    import ml_dtypes
    kern = np.asarray(kern, np.float64)[:, 0]        # (3,17,5)
    N = np.asarray(N_norm, np.float64)[:, 0]         # (3,2048,2048)
    D0 = (np.asarray(y) - np.asarray(X))[:, 0]       # (2,2048,2048) f32
    yf = np.asarray(y)[:, 0]

    n_ang = kern.shape[0]
    s = kern.sum(axis=(1, 2))                        # (3,)
    cols = [[dx for dx in range(kern.shape[2]) if np.abs(kern[a, :, dx]).sum() > 0]
            for a in range(n_ang)]

    # g_row(global row) = s / N(row, center col); 1.0 off-image
    grow_full = np.ones((n_ang, M_IMG), np.float64)
    for a in range(n_ang):
        grow_full[a] = s[a] / N[a, :, N_IMG // 2]

    # weight-index lookup shared with program builder
    widx = {}
    i = 0
    for a in range(n_ang):
        for v in range(3):
            for dx in cols[a]:
                widx[(a, v, dx)] = i
                i += 1
    n_w = i

    in_maps = []
    for core in range(BATCH * H_SHARDS):
        b, h = core // H_SHARDS, core % H_SHARDS
        gs = SH * h - 32                             # global row of slab row 0

        d0s = np.zeros((SLAB, CW), np.float32)
        r0, r1 = max(0, gs), min(M_IMG, gs + SLAB)
        d0s[r0 - gs:r1 - gs, 2:2 + N_IMG] = D0[b, r0:r1]
        yss = np.ascontiguousarray(yf[b, SH * h:SH * h + SH])   # (512, 2048) f32

        # --- banded weight matrices -------------------------------------
        # variant v: 0 applies to chunk 0, 1 to chunks 1..3, 2 to chunk 4.
        # PSUM chunk c partition m <-> slab row 112c+m <-> global gs+112c+m.
        # Only m in [8,120) is computable from the 128-row window; the other
        # partitions stay zero and are refilled by the chunk-overlap DMA.
        wts = np.zeros((n_w, 128, 128), np.float64)
        for a in range(n_ang):
            for v in range(3):
                c_of_v = {0: 0, 1: 1, 2: 4}[v]
                g_glob = gs + CH_STEP * c_of_v + np.arange(128)
                mask = (g_glob >= 0) & (g_glob < M_IMG)
                growv = np.where(mask, grow_full[a][np.clip(g_glob, 0, M_IMG - 1)], 0.0)
                for dx in cols[a]:
                    W = wts[widx[(a, v, dx)]]
                    for m in range(8, 120):
                        if mask[m]:
                            W[m - PC:m - PC + KH, m] -= growv[m] * kern[a, :, dx] / s[a]
                            if dx == 2:
                                W[m, m] += 1.0
        # pre-transpose to the SBUF layout [k, w*128+m] so the DMA is contiguous
        wtsT = np.ascontiguousarray(wts.transpose(1, 0, 2).reshape(128, n_w * 128))

        # --- edge-column factors ----------------------------------------
        # For image cols j in {0,1,2046,2047} N varies per column; the fixed
        # value is  Dnew = (1-gc)*Dold + gc*Dwrong  with gc = N(r,ctr)/N(r,j).
        # Store A = 1-gc and B = gc per (a, c): layout [k, a*40+c*8+{A:0,B:4}+j]
        gcs = np.zeros((n_ang, NCHUNK, 2, 4, 128), np.float64)
        scol = [0, 1, N_IMG - 2, N_IMG - 1]
        for a in range(n_ang):
            for c in range(NCHUNK):
                g_glob = gs + CH_STEP * c + np.arange(128)   # slab row 112c+p
                ok = (g_glob >= 0) & (g_glob < M_IMG)
                gg = np.clip(g_glob, 0, M_IMG - 1)
                for j, cc in enumerate(scol):
                    gc = np.where(ok, N[a, gg, N_IMG // 2] / N[a, gg, cc], 1.0)
                    gcs[a, c, 0, j] = 1.0 - gc
                    gcs[a, c, 1, j] = gc
        gcsT = np.ascontiguousarray(
            gcs.transpose(4, 0, 1, 2, 3).reshape(128, n_ang * NCHUNK * 8)).astype(np.float32)

        in_maps.append({"d0": d0s.astype(ml_dtypes.bfloat16), "ys": yss,
                        "wts": wtsT.astype(ml_dtypes.bfloat16), "gcs": gcsT})

    return in_maps, cols, widx, n_w


def _build_program(cols, widx, n_w):
    import concourse.bass as bass
    from concourse import mybir

    f32 = mybir.dt.float32
    bf16 = mybir.dt.bfloat16
    nc = bass.Bass("TRN2", target_bir_lowering=False)

    d0 = nc.dram_tensor("d0", [SLAB, CW], bf16, kind="ExternalInput")
    ys = nc.dram_tensor("ys", [SH, N_IMG], f32, kind="ExternalInput")
    wts = nc.dram_tensor("wts", [128, n_w * 128], bf16, kind="ExternalInput")
    gcs = nc.dram_tensor("gcs", [128, 3 * NCHUNK * 8], f32, kind="ExternalInput")
    xo = nc.dram_tensor("xo", [SH, N_IMG], f32, kind="ExternalOutput")

    n_ang = len(cols)
    ping = [nc.alloc_sbuf_tensor(f"ping{c}", [128, CW], bf16) for c in range(NCHUNK)]
    pong = [nc.alloc_sbuf_tensor(f"pong{c}", [128, CW], bf16) for c in range(NCHUNK)]
    ytile = [nc.alloc_sbuf_tensor(f"yt{c}", [128, N_IMG], f32) for c in range(NCHUNK)]
    xt = [nc.alloc_sbuf_tensor(f"xt{c}", [128, N_IMG], f32) for c in range(NCHUNK)]
    wsb = nc.alloc_sbuf_tensor("wsb", [128, n_w * 128], bf16)
    gcsb = nc.alloc_sbuf_tensor("gcsb", [128, 3 * NCHUNK * 8], f32)
    ut = [nc.alloc_sbuf_tensor(f"ut{c}", [128, 4], f32) for c in range(NCHUNK)]
    vt = [nc.alloc_sbuf_tensor(f"vt{c}", [128, 4], f32) for c in range(NCHUNK)]
    wt = [nc.alloc_sbuf_tensor(f"wt{c}", [128, 4], f32) for c in range(NCHUNK)]
    ps = [nc.alloc_psum_tensor(f"ps{i}", [128, N_IMG], f32) for i in range(2)]

    def strip_sb(t):            # cols {2,3,2048,2049} of a [128, CW] tile
        return bass.AP(t, 2, [[CW, 128], [N_IMG - 2, 2], [1, 2]])

    def strip_ps(t):            # cols {0,1,2046,2047} of a [128, 2048] psum
        return bass.AP(t, 0, [[N_IMG, 128], [N_IMG - 2, 2], [1, 2]])

    def strip_f32(t):           # cols {0,1,2046,2047} of a [128, 2048] f32 tile
        return bass.AP(t, 0, [[N_IMG, 128], [N_IMG - 2, 2], [1, 2]])

    def pad_sb(t):              # pad cols {0,1,2050,2051}
        return bass.AP(t, 0, [[CW, 128], [CW - 2, 2], [1, 2]])

    def gc_ap(a, c, which):     # A (which=0) or B (which=1) for (a, c)
        return bass.AP(gcsb, a * NCHUNK * 8 + c * 8 + which * 4,
                       [[3 * NCHUNK * 8, 128], [2, 2], [1, 2]])

    def s22(t):                 # [128,4] tile viewed as [128,2,2]
        return t[:, :].rearrange("p (s w) -> p s w", s=2)

    W_A0 = 15 * 128             # angle-0 weight cols
    W_A1 = 3 * 128

    def src_dst(a):
        return (ping, pong) if a % 2 == 0 else (pong, ping)

    def halo_cnt(a, c):         # shalo target for angle a+1 chunk c readiness
        return 16 * (8 * a + 2 * min(c + 1, 4))

    with nc.Block() as block, \
         nc.semaphore("sin") as sin, nc.semaphore("sy") as sy, \
         nc.semaphore("smem") as smem, nc.semaphore("spe") as spe, \
         nc.semaphore("sc1") as sc1, nc.semaphore("sc2") as sc2, \
         nc.semaphore("sfx") as sfx, nc.semaphore("shalo") as shalo, \
         nc.semaphore("sint") as sint, nc.semaphore("sfin") as sfin, \
         nc.semaphore("sout") as sout:

        @block.sync
        def _(sp):
            # input DMAs, priority order; each completion bumps sin by 16
            sp.dma_start(out=wsb[:, :W_A0], in_=wts[:, :W_A0]).then_inc(sin, 16)
            sp.dma_start(out=gcsb[:, :], in_=gcs[:, :]).then_inc(sin, 16)
            for c in range(NCHUNK):
                sp.dma_start(out=ping[c][:, :],
                             in_=d0[c * CH_STEP:c * CH_STEP + 128, :]).then_inc(sin, 16)
            sp.dma_start(out=wsb[:, W_A0:W_A0 + W_A1],
                         in_=wts[:, W_A0:W_A0 + W_A1]).then_inc(sin, 16)
            sp.dma_start(out=wsb[:, W_A0 + W_A1:],
                         in_=wts[:, W_A0 + W_A1:]).then_inc(sin, 16)
            # halo syncs for angles 0,1 — per-pair gating
            for a in range(n_ang - 1):
                dst = src_dst(a)[1]
                for p in range(NCHUNK - 1):
                    k = 5 * a + p + 2
                    sp.wait_ge(sc1, k)
                    sp.wait_ge(sc2, k)
                    sp.wait_ge(sfx, k)
                    sp.dma_start(out=dst[p + 1][0:8, :],
                                 in_=dst[p][112:120, :]).then_inc(shalo, 16)
                    sp.dma_start(out=dst[p][120:128, :],
                                 in_=dst[p + 1][8:16, :]).then_inc(shalo, 16)
            # output DMAs, per chunk as soon as ready
            for c in range(NCHUNK):
                o, p0, p1 = OUT_ROWS[c]
                sp.wait_ge(sfin, c + 1)
                sp.dma_start(out=xo[o:o + (p1 - p0), :],
                             in_=xt[c][p0:p1, :]).then_inc(sout, 16)
            sp.wait_ge(sout, 16 * NCHUNK)

        @block.tensor
        def _(pe):
            for a in range(n_ang):
                src = src_dst(a)[0]
                for c in range(NCHUNK):
                    g = NCHUNK * a + c
                    # --- input readiness -------------------------------
                    if a == 0:
                        pe.wait_ge(sin, 16 * (3 + c))       # wts_a0, gcs, d0[0..c]
                    else:
                        if c == 0:
                            pe.wait_ge(sin, 16 * (7 + a))   # this angle's weights
                            if a == 1:
                                pe.wait_ge(smem, NCHUNK)    # pong pad cols zeroed
                        pe.wait_ge(shalo, halo_cnt(a - 1, c))
                    # --- psum bank free --------------------------------
                    if g >= 2:
                        if a == 2 and c >= 2:
                            pe.wait_ge(sfin, c - 1)         # DVE consumed (2,c-2)
                        else:
                            pe.wait_ge(sc1, g - 1)
                            pe.wait_ge(sc2, g - 1)
                            pe.wait_ge(sfx, g - 1)
                    v = {0: 0, NCHUNK - 1: 2}.get(c, 1)
                    dxs = cols[a]
                    for i, dx in enumerate(dxs):
                        wi = widx[(a, v, dx)]
                        for nt in range(4):
                            mm = pe.matmul(ps[g % 2][:, nt * 512:(nt + 1) * 512],
                                           lhsT=wsb[:, wi * 128:(wi + 1) * 128],
                                           rhs=src[c][:, nt * 512 + dx: nt * 512 + dx + 512],
                                           start=(i == 0), stop=(i == len(dxs) - 1))
                            if i == len(dxs) - 1 and nt == 3:
                                mm.then_inc(spe, 1)

        @block.scalar
        def _(act):
            for a in range(n_ang - 1):                      # final angle: no copy
                dst = src_dst(a)[1]
                for c in range(NCHUNK):
                    g = NCHUNK * a + c
                    act.wait_ge(spe, g + 1)
                    act.copy(out=dst[c][:, 4:CSPLIT + 2],
                             in_=ps[g % 2][:, 2:CSPLIT]).then_inc(sc1, 1)

        @block.gpsimd
        def _(gp):
            for c in range(NCHUNK):
                o, p0, p1 = OUT_ROWS[c]
                gp.dma_start(out=ytile[c][p0:p1, :],
                             in_=ys[o:o + (p1 - p0), :]).then_inc(sy, 16)

        @block.vector
        def _(dve):
            ki = 0
            for c in range(NCHUNK):
                dve.memset(pad_sb(pong[c]), 0.0).then_inc(smem, 1)
            for a in range(n_ang):
                src, dst = src_dst(a)
                last = (a == n_ang - 1)
                for c in range(NCHUNK):
                    g = NCHUNK * a + c
                    # u = A * strip(src) — ready as soon as src state-a is
                    if a == 0:
                        dve.wait_ge(sin, 16 * (3 + c))      # gcs + d0[0..c]
                    else:
                        dve.wait_ge(shalo, halo_cnt(a - 1, c))
                    dve.tensor_mul(s22(ut[c]), strip_sb(src[c]),
                                   gc_ap(a, c, 0)).then_inc(sint, 1)
                    ki += 1
                    # v = B * strip(psum)
                    dve.wait_ge(spe, g + 1)
                    dve.tensor_mul(s22(vt[c]), strip_ps(ps[g % 2]),
                                   gc_ap(a, c, 1)).then_inc(sint, 1)
                    ki += 1
                    if not last:
                        # right-hand share of the PSUM->SBUF copy
                        dve.tensor_copy(out=dst[c][:, CSPLIT + 2:N_IMG],
                                        in_=ps[g % 2][:, CSPLIT:N_IMG - 2]).then_inc(sc2, 1)
                    dve.wait_ge(sint, ki)
                    if not last:
                        # strip(dst) = u + v
                        dve.tensor_add(strip_sb(dst[c]), s22(ut[c]),
                                       s22(vt[c])).then_inc(sfx, 1)
                    else:
                        # w = u + v;  xt = y - psum;  strip(xt) = strip(y) - w
                        dve.tensor_add(s22(wt[c]), s22(ut[c]),
                                       s22(vt[c])).then_inc(sint, 1)
                        ki += 1
                        dve.wait_ge(sy, 16 * (c + 1))
                        dve.tensor_sub(xt[c][:, :], ytile[c][:, :],
                                       ps[g % 2][:, :]).then_inc(sint, 1)
                        ki += 1
                        dve.wait_ge(sint, ki)
                        dve.tensor_sub(strip_f32(xt[c]), strip_f32(ytile[c]),
                                       s22(wt[c])).then_inc(sfin, 1)
    return nc


_LAST = None  # BassKernelResults of the most recent run (for test harness)


def kernel(X, y, kernel, N_norm):
    global _LAST
    from concourse.bass_utils import run_bass_kernel_spmd

    in_maps, cols, widx, n_w = _host_prep(X, y, kernel, N_norm)
    nc = _build_program(cols, widx, n_w)
    res = run_bass_kernel_spmd(nc, in_maps, list(range(BATCH * H_SHARDS)))
    _LAST = res

    out = np.empty((BATCH, 1, M_IMG, N_IMG), np.float32)
    for core in range(BATCH * H_SHARDS):
        b, h = core // H_SHARDS, core % H_SHARDS
        out[b, 0, SH * h:SH * h + SH, :] = res.results[core]["xo"]
    return out
